# revision 7
# baseline (speedup 1.0000x reference)
"""Trainium2 Bass kernel for nn_K_attention_ex (gaussian-kernel residual attention).

Reference computation (per batch sample b):
    sq_i   = ||x_i||^2
    G      = x @ x^T                      (T,T) gram
    sqdist = relu(sq_i + sq_j - 2 G)
    K      = exp(-sqdist * r + m) * (1 - eye)
    out    = x + K @ x

Algebraic restructuring (exact up to fp rounding):
    out = (1-beta)*x + beta * e ⊙_row ( E @ (e ⊙_row x) ),
    e = exp(-r*sq), beta = exp(m), E = exp(2 r G).

Sharding: data-parallel over batch B=16 across 8 NeuronCores (2 samples each).

v2 layout: both samples share the PE array via tile_position packing.
  xT_both (128, T): partitions 0-63 = sample0^T, 64-127 = sample1^T.
  Gram: K=64 row-tiled matmuls at row positions 0 / 64 -> run concurrently.
  Y:    M=64 col-tiled matmuls at col positions 0 / 64 -> run concurrently;
        YT_both (128, T) psum: partitions 0-63 = YT(s0), 64-127 = YT(s1).
  PSUM: YT_both 4 banks + 2x G(128,1024) ping-pong 4 banks = 8.
  ACT (exp) is the bottleneck engine: 64 calls of N=1024 ~= 61us.
"""

import numpy as np

import concourse.bass as bass
import concourse.tile as tile
from concourse import bacc, mybir
from concourse.bass_utils import run_bass_kernel_spmd
from concourse.masks import make_identity

F32 = mybir.dt.float32
F32R = mybir.dt.float32r  # fp32 data, PE fast-fp32 matmul mode (1 cyc/col @ N>=256)
BF16 = mybir.dt.bfloat16
AF = mybir.ActivationFunctionType
B, T, C = 16, 2048, 64
N_CORES = 8
BPC = B // N_CORES          # samples per core
NK = T // 128               # 16 row-blocks of 128


def build_nc(reps=1, stages='all'):
    nc = bacc.Bacc("TRN2", target_bir_lowering=False, debug=False, num_devices=N_CORES)
    x_in = nc.dram_tensor("x", [BPC, T, C], F32, kind="ExternalInput")
    r_in = nc.dram_tensor("r_sigma", [1], F32, kind="ExternalInput")
    m_in = nc.dram_tensor("margin", [1], F32, kind="ExternalInput")
    o_out = nc.dram_tensor("out", [BPC, T, C], F32, kind="ExternalOutput")

    with tile.TileContext(nc) as tc:
        if reps == 1:
            _body(tc, o_out.ap(), x_in.ap(), r_in.ap(), m_in.ap(), stages)
        else:
            with tc.For_i(0, reps, 1):
                _body(tc, o_out.ap(), x_in.ap(), r_in.ap(), m_in.ap(), stages)
    nc.compile()
    return nc


def _body(tc, out_ap, x_ap, r_ap, m_ap, stages='all'):
    nc = tc.nc
    with (
        tc.tile_pool(name="consts", bufs=1) as consts,
        tc.tile_pool(name="sx", bufs=2) as sx,
        tc.tile_pool(name="epool", bufs=4) as epool,
        tc.tile_pool(name="psG", bufs=2, space="PSUM") as psG,
        tc.tile_pool(name="psY", bufs=1, space="PSUM") as psY,
    ):
        # ---- one-time constants ----
        ident = consts.tile([128, 128], F32)
        make_identity(nc, ident)
        rb = consts.tile([128, 1], F32)
        nc.gpsimd.dma_start(out=rb, in_=r_ap.to_broadcast((128, 1)))
        mb = consts.tile([128, 1], F32)
        nc.gpsimd.dma_start(out=mb, in_=m_ap.to_broadcast((128, 1)))
        negr = consts.tile([128, 1], F32)
        nc.vector.tensor_scalar_mul(out=negr, in0=rb, scalar1=-1.0)
        s2r = consts.tile([128, 1], F32)
        nc.vector.tensor_scalar_mul(out=s2r, in0=rb, scalar1=2.0)
        beta = consts.tile([128, 1], F32)
        nc.scalar.activation(out=beta, in_=mb, func=AF.Exp)
        alpha = consts.tile([128, 1], F32)  # 1 - beta
        nc.vector.tensor_scalar(
            out=alpha, in0=beta, scalar1=-1.0, scalar2=1.0,
            op0=mybir.AluOpType.mult, op1=mybir.AluOpType.add,
        )

        # prefetch both samples' inputs, interleaved in the free dim:
        # x_sb2 (128, NK, 128) with sample s at columns 64s..64s+64. One
        # 128-wide PE transpose then yields both samples stacked across
        # partitions (s0 -> rows 0-63, s1 -> rows 64-127).
        x_sb2 = sx.tile([128, NK, 2 * C], F32, name="x_sb2")
        for s in range(BPC):
            xv = x_ap[s].rearrange("(p k) c -> p k c", p=128)
            nc.sync.dma_start(out=x_sb2[:, 0:8, 64 * s : 64 * s + 64], in_=xv[:, 0:8, :])
            nc.gpsimd.dma_start(
                out=x_sb2[:, 8:NK, 64 * s : 64 * s + 64], in_=xv[:, 8:NK, :]
            )

        # ---- xT_both (128, T): partitions 64s..64s+64 = sample s transposed ----
        xT = sx.tile([128, T], F32R, name="xT_both")
        for g in range(4):
            xtr = psG.tile([128, 4, 128], F32, tag="G", name=f"xtr_{g}")
            for kk in range(4):
                k = 4 * g + kk
                nc.tensor.transpose(
                    out=xtr[:, kk, :], in_=x_sb2[:, k, :], identity=ident,
                )
            nc.vector.tensor_copy(
                out=xT[:, 512 * g : 512 * (g + 1)],
                in_=xtr.rearrange("p a b -> p (a b)"),
            )

        # ---- per-row scalars, both samples ----
        xsq2 = sx.tile([128, NK, 2 * C], F32, name="xsq2")
        nc.vector.tensor_mul(xsq2, x_sb2, x_sb2)
        ax2 = sx.tile([128, NK, 2 * C], F32, name="ax2")
        nc.vector.tensor_scalar_mul(out=ax2, in0=x_sb2, scalar1=alpha)
        fs, xss = [], []
        for s in range(BPC):
            sq = sx.tile([128, NK], F32, name=f"sq_{s}")
            nc.vector.reduce_sum(
                out=sq, in_=xsq2[:, :, 64 * s : 64 * s + 64],
                axis=mybir.AxisListType.X,
            )
            e = sx.tile([128, NK], F32, name=f"e_{s}")
            nc.scalar.activation(out=e, in_=sq, func=AF.Exp, scale=negr)
            f = sx.tile([128, NK], F32, name=f"f_{s}")
            nc.vector.tensor_scalar_mul(out=f, in0=e, scalar1=beta)
            xs_t = sx.tile([128, NK, C], BF16, name=f"xs_{s}")
            for k in range(NK):
                nc.vector.tensor_scalar_mul(
                    out=xs_t[:, k, :],
                    in0=x_sb2[:, k, 64 * s : 64 * s + 64],
                    scalar1=e[:, k : k + 1],
                )
            fs.append(f)
            xss.append(xs_t)

        # ---- main loop: j row-blocks x (sample, half) phases ----
        # Per phase p=(s,h): gram 2 MMs N=512 (row-tile 64s) -> exp N=1024 ->
        # Y 2 MMs N=512 (col-tile 64s). Software-pipelined: emit gram(p+1)
        # before Y(p) so PE always has ACT-independent work queued.
        YT = psY.tile([128, T], F32, name='YT_both')

        phases = [(j, s, h) for j in range(NK) for h in range(2) for s in range(BPC)]

        def emit_gram(p):
            j, s, h = p
            G = psG.tile([128, 1024], F32, tag="G", name=f"G_{s}_{j}_{h}")
            lhsT_g = xT[64 * s : 64 * s + 64, 128 * j : 128 * (j + 1)]
            for q in range(2):
                n0 = 1024 * h + 512 * q
                nc.tensor.matmul(
                    out=G[:, 512 * q : 512 * (q + 1)],
                    lhsT=lhsT_g,
                    rhs=xT[64 * s : 64 * s + 64, n0 : n0 + 512],
                    start=True,
                    stop=True,
                )
            return G

        def emit_exp(G, p):
            j, s, h = p
            E = epool.tile([128, 1024], BF16, tag="E", name=f"E_{s}_{j}_{h}")
            nc.scalar.activation(out=E, in_=G, func=AF.Exp, scale=s2r)
            return E

        def emit_y(E, p):
            j, s, h = p
            for q in range(2):
                n0 = 1024 * h + 512 * q
                nc.tensor.matmul(
                    out=YT[64 * s : 64 * s + 64, n0 : n0 + 512],
                    lhsT=xss[s][:, j, :],
                    rhs=E[:, 512 * q : 512 * (q + 1)],
                    start=(j == 0),
                    stop=(j == NK - 1),
                )

        G = emit_gram(phases[0])
        prev = None  # (E, phase)
        for i, p in enumerate(phases):
            E = emit_exp(G, p)
            if i + 1 < len(phases):
                G = emit_gram(phases[i + 1])
            if prev is not None:
                emit_y(*prev)
            prev = (E, p)
        emit_y(*prev)

        # ---- back to natural layout + combine (pipelined per 4-block chunk) ----
        # YTsb rows 0-63 = YT(s0), rows 64-127 = YT(s1); one (128,128)
        # transpose per row-block k yields ytr[:, 0:64] = Y(s0) natural,
        # ytr[:, 64:128] = Y(s1) natural.
        YTsb = sx.tile([128, T], F32, name="YTsb_both")
        outsbs = [
            sx.tile([128, NK, C], F32, tag=f"outsb{s}", name=f"outsb_{s}")
            for s in range(BPC)
        ]
        ovs = [out_ap[s].rearrange("(p k) c -> p k c", p=128) for s in range(BPC)]
        dma_legs = [nc.sync, nc.gpsimd, nc.scalar, nc.sync]
        for g in range(4):
            nc.vector.tensor_copy(
                out=YTsb[:, 512 * g : 512 * (g + 1)],
                in_=YT[:, 512 * g : 512 * (g + 1)],
            )
            ytr = psG.tile([128, 4, 2 * C], F32, tag="G", name=f"ytr_{g}")
            for kk in range(4):
                k = 4 * g + kk
                nc.tensor.transpose(
                    out=ytr[:, kk, :],
                    in_=YTsb[:, 128 * k : 128 * (k + 1)],
                    identity=ident,
                )
            for s in range(BPC):
                for kk in range(4):
                    k = 4 * g + kk
                    nc.vector.scalar_tensor_tensor(
                        out=outsbs[s][:, k, :],
                        in0=ytr[:, kk, 64 * s : 64 * s + 64],
                        scalar=fs[s][:, k : k + 1],
                        in1=ax2[:, k, 64 * s : 64 * s + 64],
                        op0=mybir.AluOpType.mult,
                        op1=mybir.AluOpType.add,
                    )
            for s in range(BPC):
                dma_legs[(2 * g + s) % 4].dma_start(
                    out=ovs[s][:, 4 * g : 4 * (g + 1), :],
                    in_=outsbs[s][:, 4 * g : 4 * (g + 1), :],
                )


_NC_CACHE = {}


def _get_nc(reps=1, stages='all'):
    key = (reps, stages)
    if key not in _NC_CACHE:
        _NC_CACHE[key] = build_nc(reps, stages)
    return _NC_CACHE[key]


def _run(x, r_sigma, margin, trace=False, reps=1, stages='all'):
    nc = _get_nc(reps, stages)
    x = np.ascontiguousarray(np.asarray(x, dtype=np.float32))
    r_sigma = np.ascontiguousarray(np.asarray(r_sigma, dtype=np.float32))
    margin = np.ascontiguousarray(np.asarray(margin, dtype=np.float32))
    in_maps = [
        {
            "x": np.ascontiguousarray(x[c * BPC : (c + 1) * BPC]),
            "r_sigma": r_sigma,
            "margin": margin,
        }
        for c in range(N_CORES)
    ]
    res = run_bass_kernel_spmd(nc, in_maps, core_ids=list(range(N_CORES)), trace=trace)
    out = np.concatenate([res.results[c]["out"] for c in range(N_CORES)], axis=0)
    return out, res


def kernel(x, r_sigma, margin):
    out, _ = _run(x, r_sigma, margin, trace=False)
    return out


# revision 8
# speedup vs baseline: 1.4840x; 1.4840x over previous
"""Trainium2 Bass kernel for nn_K_attention_ex (gaussian-kernel residual attention).

Reference computation (per batch sample b):
    sq_i   = ||x_i||^2
    G      = x @ x^T                      (T,T) gram
    sqdist = relu(sq_i + sq_j - 2 G)
    K      = exp(-sqdist * r + m) * (1 - eye)
    out    = x + K @ x

Algebraic restructuring (exact up to fp rounding):
    out = (1-beta)*x + beta * e ⊙_row ( E @ (e ⊙_row x) ),
    e = exp(-r*sq), beta = exp(m), E = exp(2 r G).

Sharding: data-parallel over batch B=16 across 8 NeuronCores (2 samples each).

v2d: single merged phase loop over both samples (ACT never idles between
samples; exp is the bottleneck engine at ~1 elem/cycle/lane).
  x_sb2 (128, NK, 128): sample s interleaved at free cols 64s..64s+64, so
      one 128-wide PE transpose serves both samples.
  xT_s (128, T) f32r per sample: sample s's x^T in partitions 64s..64s+64,
      other half zeroed -> K=128 matmul stays on the PE fast-fp32 path
      (measured: K=64 f32r is 3x slower; K=128 f32r N=512 = 196ns).
  Y: bf16 col-tiled pairs into YT_both (128, T) psum: YT(s) at partitions
      64s..64s+64 (f32r col-tiling is rejected by codegen; bf16 works).
  PSUM: YT_both 4 banks + 2x G(128,1024) ping-pong = 8 banks.
"""

import numpy as np

import concourse.bass as bass
import concourse.tile as tile
from concourse import bacc, mybir
from concourse.bass_utils import run_bass_kernel_spmd
from concourse.masks import make_identity

F32 = mybir.dt.float32
F32R = mybir.dt.float32r
BF16 = mybir.dt.bfloat16
AF = mybir.ActivationFunctionType
B, T, C = 16, 2048, 64
N_CORES = 8
BPC = B // N_CORES          # samples per core
NK = T // 128               # 16 row-blocks of 128


def build_nc(reps=1, stages='all'):
    nc = bacc.Bacc("TRN2", target_bir_lowering=False, debug=False, num_devices=N_CORES)
    x_in = nc.dram_tensor("x", [BPC, T, C], F32, kind="ExternalInput")
    r_in = nc.dram_tensor("r_sigma", [1], F32, kind="ExternalInput")
    m_in = nc.dram_tensor("margin", [1], F32, kind="ExternalInput")
    o_out = nc.dram_tensor("out", [BPC, T, C], F32, kind="ExternalOutput")

    with tile.TileContext(nc) as tc:
        if reps == 1:
            _body(tc, o_out.ap(), x_in.ap(), r_in.ap(), m_in.ap(), stages)
        else:
            with tc.For_i(0, reps, 1):
                _body(tc, o_out.ap(), x_in.ap(), r_in.ap(), m_in.ap(), stages)
    nc.compile()
    return nc


def _body(tc, out_ap, x_ap, r_ap, m_ap, stages='all'):
    nc = tc.nc
    with (
        tc.tile_pool(name="consts", bufs=1) as consts,
        tc.tile_pool(name="sx", bufs=2) as sx,
        tc.tile_pool(name="epool", bufs=4) as epool,
        tc.tile_pool(name="psG", bufs=2, space="PSUM") as psG,
        tc.tile_pool(name="psY", bufs=1, space="PSUM") as psY,
    ):
        # ---- one-time constants ----
        ident = consts.tile([128, 128], F32)
        make_identity(nc, ident)
        rb = consts.tile([128, 1], F32)
        nc.gpsimd.dma_start(out=rb, in_=r_ap.to_broadcast((128, 1)))
        mb = consts.tile([128, 1], F32)
        nc.gpsimd.dma_start(out=mb, in_=m_ap.to_broadcast((128, 1)))
        negr = consts.tile([128, 1], F32)
        nc.vector.tensor_scalar_mul(out=negr, in0=rb, scalar1=-1.0)
        s2r = consts.tile([128, 1], F32)
        nc.vector.tensor_scalar_mul(out=s2r, in0=rb, scalar1=2.0)
        beta = consts.tile([128, 1], F32)
        nc.scalar.activation(out=beta, in_=mb, func=AF.Exp)
        alpha = consts.tile([128, 1], F32)  # 1 - beta
        nc.vector.tensor_scalar(
            out=alpha, in0=beta, scalar1=-1.0, scalar2=1.0,
            op0=mybir.AluOpType.mult, op1=mybir.AluOpType.add,
        )

        # ---- input: x_sb2 (128, NK, 128), sample s at free cols 64s.. ----
        x_sb2 = sx.tile([128, NK, 2 * C], F32, name="x_sb2")
        for s in range(BPC):
            xv = x_ap[s].rearrange("(p k) c -> p k c", p=128)
            nc.sync.dma_start(out=x_sb2[:, 0:4, 64 * s : 64 * s + 64], in_=xv[:, 0:4, :])
            nc.scalar.dma_start(
                out=x_sb2[:, 4:8, 64 * s : 64 * s + 64], in_=xv[:, 4:8, :]
            )
            nc.gpsimd.dma_start(
                out=x_sb2[:, 8:NK, 64 * s : 64 * s + 64], in_=xv[:, 8:NK, :]
            )

        # ---- xT_s (128, T) f32r: sample s data at partitions 64s..,
        #      other 64 partitions zero (K=128 keeps the fast-fp32 path) ----
        xTs = []
        for s in range(BPC):
            xT = sx.tile([128, T], F32R, name=f"xT_{s}")
            nc.vector.tensor_scalar_mul(
                out=xT[64 * (1 - s) : 64 * (1 - s) + 64, :],
                in0=ident[64 * (1 - s) : 64 * (1 - s) + 64, 0:1].to_broadcast((64, T)),
                scalar1=0.0,
            )
            xTs.append(xT)
        for g in range(4):
            xtr = psG.tile([128, 4, 128], F32, tag="G", name=f"xtr_{g}")
            for kk in range(4):
                k = 4 * g + kk
                nc.tensor.transpose(
                    out=xtr[:, kk, :], in_=x_sb2[:, k, :], identity=ident,
                )
            for s in range(BPC):
                nc.vector.tensor_copy(
                    out=xTs[s][64 * s : 64 * s + 64, 512 * g : 512 * (g + 1)],
                    in_=xtr[64 * s : 64 * s + 64, :, :].rearrange("p a b -> p (a b)"),
                )

        # ---- per-row scalars, both samples ----
        xsq2 = sx.tile([128, NK, 2 * C], F32, name="xsq2")
        nc.vector.tensor_mul(xsq2, x_sb2, x_sb2)
        ax2 = sx.tile([128, NK, 2 * C], F32, name="ax2")
        nc.vector.tensor_scalar_mul(out=ax2, in0=x_sb2, scalar1=alpha)
        fs, xss = [], []
        for s in range(BPC):
            sq = sx.tile([128, NK], F32, name=f"sq_{s}")
            nc.vector.reduce_sum(
                out=sq, in_=xsq2[:, :, 64 * s : 64 * s + 64],
                axis=mybir.AxisListType.X,
            )
            e = sx.tile([128, NK], F32, name=f"e_{s}")
            nc.scalar.activation(out=e, in_=sq, func=AF.Exp, scale=negr)
            f = sx.tile([128, NK], F32, name=f"f_{s}")
            nc.vector.tensor_scalar_mul(out=f, in0=e, scalar1=beta)
            xs_t = sx.tile([128, NK, C], BF16, name=f"xs_{s}")
            for k in range(NK):
                nc.vector.tensor_scalar_mul(
                    out=xs_t[:, k, :],
                    in0=x_sb2[:, k, 64 * s : 64 * s + 64],
                    scalar1=e[:, k : k + 1],
                )
            fs.append(f)
            xss.append(xs_t)

        # ---- main loop: phases (j, h, s); gram K=128 f32r, Y bf16 col-tiled.
        # Software-pipelined: emit gram(p+1) before Y(p).
        YT = psY.tile([128, T], F32, name='YT_both')
        phases = [(j, s, h) for j in range(NK) for h in range(2) for s in range(BPC)]

        def emit_gram(p):
            j, s, h = p
            G = psG.tile([128, 1024], F32, tag="G", name=f"G_{s}_{j}_{h}")
            lhsT_g = xTs[s][:, 128 * j : 128 * (j + 1)]
            for q in range(2):
                n0 = 1024 * h + 512 * q
                nc.tensor.matmul(
                    out=G[:, 512 * q : 512 * (q + 1)],
                    lhsT=lhsT_g,
                    rhs=xTs[s][:, n0 : n0 + 512],
                    start=True,
                    stop=True,
                )
            return G

        def emit_exp(G, p):
            j, s, h = p
            E = epool.tile([128, 1024], BF16, tag="E", name=f"E_{s}_{j}_{h}")
            nc.scalar.activation(out=E, in_=G, func=AF.Exp, scale=s2r)
            return E

        def emit_y(E, p):
            j, s, h = p
            for q in range(2):
                n0 = 1024 * h + 512 * q
                nc.tensor.matmul(
                    out=YT[64 * s : 64 * s + 64, n0 : n0 + 512],
                    lhsT=xss[s][:, j, :],
                    rhs=E[:, 512 * q : 512 * (q + 1)],
                    start=(j == 0),
                    stop=(j == NK - 1),
                )

        G = emit_gram(phases[0])
        prev = None
        for i, p in enumerate(phases):
            E = emit_exp(G, p)
            if i + 1 < len(phases):
                G = emit_gram(phases[i + 1])
            if prev is not None:
                emit_y(*prev)
            prev = (E, p)
        emit_y(*prev)

        # ---- epilogue: YT -> natural + combine + store ----
        YTsb = sx.tile([128, T], F32, name="YTsb_both")
        outsbs = [
            sx.tile([128, NK, C], F32, tag=f"outsb{s}", name=f"outsb_{s}")
            for s in range(BPC)
        ]
        ovs = [out_ap[s].rearrange("(p k) c -> p k c", p=128) for s in range(BPC)]
        dma_legs = [nc.sync, nc.gpsimd, nc.scalar, nc.sync]
        for g in range(4):
            nc.vector.tensor_copy(
                out=YTsb[:, 512 * g : 512 * (g + 1)],
                in_=YT[:, 512 * g : 512 * (g + 1)],
            )
            ytr = psG.tile([128, 4, 2 * C], F32, tag="G", name=f"ytr_{g}")
            for kk in range(4):
                k = 4 * g + kk
                nc.tensor.transpose(
                    out=ytr[:, kk, :],
                    in_=YTsb[:, 128 * k : 128 * (k + 1)],
                    identity=ident,
                )
            for s in range(BPC):
                for kk in range(4):
                    k = 4 * g + kk
                    nc.vector.scalar_tensor_tensor(
                        out=outsbs[s][:, k, :],
                        in0=ytr[:, kk, 64 * s : 64 * s + 64],
                        scalar=fs[s][:, k : k + 1],
                        in1=ax2[:, k, 64 * s : 64 * s + 64],
                        op0=mybir.AluOpType.mult,
                        op1=mybir.AluOpType.add,
                    )
            for s in range(BPC):
                dma_legs[(2 * g + s) % 4].dma_start(
                    out=ovs[s][:, 4 * g : 4 * (g + 1), :],
                    in_=outsbs[s][:, 4 * g : 4 * (g + 1), :],
                )


_NC_CACHE = {}


def _get_nc(reps=1, stages='all'):
    key = (reps, stages)
    if key not in _NC_CACHE:
        _NC_CACHE[key] = build_nc(reps, stages)
    return _NC_CACHE[key]


def _run(x, r_sigma, margin, trace=False, reps=1, stages='all'):
    nc = _get_nc(reps, stages)
    x = np.ascontiguousarray(np.asarray(x, dtype=np.float32))
    r_sigma = np.ascontiguousarray(np.asarray(r_sigma, dtype=np.float32))
    margin = np.ascontiguousarray(np.asarray(margin, dtype=np.float32))
    in_maps = [
        {
            "x": np.ascontiguousarray(x[c * BPC : (c + 1) * BPC]),
            "r_sigma": r_sigma,
            "margin": margin,
        }
        for c in range(N_CORES)
    ]
    res = run_bass_kernel_spmd(nc, in_maps, core_ids=list(range(N_CORES)), trace=trace)
    out = np.concatenate([res.results[c]["out"] for c in range(N_CORES)], axis=0)
    return out, res


def kernel(x, r_sigma, margin):
    out, _ = _run(x, r_sigma, margin, trace=False)
    return out


# revision 9
# speedup vs baseline: 1.9882x; 1.3398x over previous
"""Trainium2 Bass kernel for nn_K_attention_ex (gaussian-kernel residual attention).

Reference computation (per batch sample b):
    sq_i   = ||x_i||^2
    G      = x @ x^T                      (T,T) gram
    sqdist = relu(sq_i + sq_j - 2 G)
    K      = exp(-sqdist * r + m) * (1 - eye)
    out    = x + K @ x

Algebraic restructuring (exact up to fp rounding):
    out = (1-beta)*x + beta * e ⊙_row ( E @ (e ⊙_row x) ),
    e = exp(-r*sq), beta = exp(m), E = exp(2 r G).

Sharding: data-parallel over batch B=16 across 8 NeuronCores (2 samples each).

v2d: single merged phase loop over both samples (ACT never idles between
samples; exp is the bottleneck engine at ~1 elem/cycle/lane).
  x_sb2 (128, NK, 128): sample s interleaved at free cols 64s..64s+64, so
      one 128-wide PE transpose serves both samples.
  xT_s (128, T) f32r per sample: sample s's x^T in partitions 64s..64s+64,
      other half zeroed -> K=128 matmul stays on the PE fast-fp32 path
      (measured: K=64 f32r is 3x slower; K=128 f32r N=512 = 196ns).
  Y: bf16 col-tiled pairs into YT_both (128, T) psum: YT(s) at partitions
      64s..64s+64 (f32r col-tiling is rejected by codegen; bf16 works).
  PSUM: YT_both 4 banks + 2x G(128,1024) ping-pong = 8 banks.
"""

import numpy as np

import concourse.bass as bass
import concourse.tile as tile
from concourse import bacc, mybir
from concourse.bass_utils import run_bass_kernel_spmd
from concourse.masks import make_identity

F32 = mybir.dt.float32
F32R = mybir.dt.float32r
BF16 = mybir.dt.bfloat16
AF = mybir.ActivationFunctionType
B, T, C = 16, 2048, 64
N_CORES = 8
BPC = B // N_CORES          # samples per core
NK = T // 128               # 16 row-blocks of 128


def build_nc(reps=1, stages='all'):
    nc = bacc.Bacc("TRN2", target_bir_lowering=False, debug=False, num_devices=N_CORES)
    x_in = nc.dram_tensor("x", [BPC, T, C], F32, kind="ExternalInput")
    r_in = nc.dram_tensor("r_sigma", [1], F32, kind="ExternalInput")
    m_in = nc.dram_tensor("margin", [1], F32, kind="ExternalInput")
    o_out = nc.dram_tensor("out", [BPC, T, C], F32, kind="ExternalOutput")

    with tile.TileContext(nc) as tc:
        if reps == 1:
            _body(tc, o_out.ap(), x_in.ap(), r_in.ap(), m_in.ap(), stages)
        else:
            with tc.For_i(0, reps, 1):
                _body(tc, o_out.ap(), x_in.ap(), r_in.ap(), m_in.ap(), stages)
    nc.compile()
    return nc


def _body(tc, out_ap, x_ap, r_ap, m_ap, stages='all'):
    nc = tc.nc
    with (
        tc.tile_pool(name="consts", bufs=1) as consts,
        tc.tile_pool(name="sx", bufs=2) as sx,
        tc.tile_pool(name="epool", bufs=4) as epool,
        tc.tile_pool(name="psG", bufs=2, space="PSUM") as psG,
        tc.tile_pool(name="psY", bufs=1, space="PSUM") as psY,
    ):
        # ---- one-time constants ----
        ident = consts.tile([128, 128], F32)
        make_identity(nc, ident)
        rb = consts.tile([128, 1], F32)
        nc.gpsimd.dma_start(out=rb, in_=r_ap.to_broadcast((128, 1)))
        mb = consts.tile([128, 1], F32)
        nc.gpsimd.dma_start(out=mb, in_=m_ap.to_broadcast((128, 1)))
        negr = consts.tile([128, 1], F32)
        nc.vector.tensor_scalar_mul(out=negr, in0=rb, scalar1=-1.0)
        s2r = consts.tile([128, 1], F32)
        nc.vector.tensor_scalar_mul(out=s2r, in0=rb, scalar1=2.0)
        beta = consts.tile([128, 1], F32)
        nc.scalar.activation(out=beta, in_=mb, func=AF.Exp)
        alpha = consts.tile([128, 1], F32)  # 1 - beta
        nc.vector.tensor_scalar(
            out=alpha, in0=beta, scalar1=-1.0, scalar2=1.0,
            op0=mybir.AluOpType.mult, op1=mybir.AluOpType.add,
        )

        # ---- input: x_sb2 (128, NK, 128), sample s at free cols 64s.. ----
        x_sb2 = sx.tile([128, NK, 2 * C], F32, name="x_sb2")
        for s in range(BPC):
            xv = x_ap[s].rearrange("(p k) c -> p k c", p=128)
            nc.sync.dma_start(out=x_sb2[:, 0:4, 64 * s : 64 * s + 64], in_=xv[:, 0:4, :])
            nc.scalar.dma_start(
                out=x_sb2[:, 4:8, 64 * s : 64 * s + 64], in_=xv[:, 4:8, :]
            )
            nc.gpsimd.dma_start(
                out=x_sb2[:, 8:NK, 64 * s : 64 * s + 64], in_=xv[:, 8:NK, :]
            )

        # ---- xT_s (128, T) f32r: sample s data at partitions 64s..,
        #      other 64 partitions zero (K=128 keeps the fast-fp32 path) ----
        xTs = []
        for s in range(BPC):
            xT = sx.tile([128, T], F32R, name=f"xT_{s}")
            nc.vector.tensor_scalar_mul(
                out=xT[64 * (1 - s) : 64 * (1 - s) + 64, :],
                in0=ident[64 * (1 - s) : 64 * (1 - s) + 64, 0:1].to_broadcast((64, T)),
                scalar1=0.0,
            )
            xTs.append(xT)
        for g in range(4):
            xtr = psG.tile([128, 4, 128], F32, tag="G", name=f"xtr_{g}")
            for kk in range(4):
                k = 4 * g + kk
                nc.tensor.transpose(
                    out=xtr[:, kk, :], in_=x_sb2[:, k, :], identity=ident,
                )
            for s in range(BPC):
                nc.vector.tensor_copy(
                    out=xTs[s][64 * s : 64 * s + 64, 512 * g : 512 * (g + 1)],
                    in_=xtr[64 * s : 64 * s + 64, :, :].rearrange("p a b -> p (a b)"),
                )

        # ---- per-row scalars, both samples ----
        xsq2 = sx.tile([128, NK, 2 * C], F32, name="xsq2")
        nc.vector.tensor_mul(xsq2, x_sb2, x_sb2)
        ax2 = sx.tile([128, NK, 2 * C], F32, name="ax2")
        nc.vector.tensor_scalar_mul(out=ax2, in0=x_sb2, scalar1=alpha)
        fs, xss = [], []
        for s in range(BPC):
            sq = sx.tile([128, NK], F32, name=f"sq_{s}")
            nc.vector.reduce_sum(
                out=sq, in_=xsq2[:, :, 64 * s : 64 * s + 64],
                axis=mybir.AxisListType.X,
            )
            e = sx.tile([128, NK], F32, name=f"e_{s}")
            nc.scalar.activation(out=e, in_=sq, func=AF.Exp, scale=negr)
            f = sx.tile([128, NK], F32, name=f"f_{s}")
            nc.vector.tensor_scalar_mul(out=f, in0=e, scalar1=beta)
            xs_t = sx.tile([128, NK, C], BF16, name=f"xs_{s}")
            for k in range(NK):
                nc.vector.tensor_scalar_mul(
                    out=xs_t[:, k, :],
                    in0=x_sb2[:, k, 64 * s : 64 * s + 64],
                    scalar1=e[:, k : k + 1],
                )
            fs.append(f)
            xss.append(xs_t)

        # ---- main loop: phases (j, h) covering both samples; gram K=128
        # f32r (fast-fp32 path, 196ns/MM). Y matmuls are bf16 col-tiled and
        # MUST be emitted as adjacent (s0, s1) pairs: measured 931ns for an
        # isolated M=64 bf16 MM vs ~50ns for an adjacent col-tiled pair.
        # Software-pipelined: emit gram(p+1) before Y(p).
        YT = psY.tile([128, T], F32, name='YT_both')
        phases = [(j, h) for j in range(NK) for h in range(2)]

        def emit_gram(p):
            j, h = p
            Gs = []
            for s in range(BPC):
                G = psG.tile([128, 1024], F32, tag="G", name=f"G_{s}_{j}_{h}")
                lhsT_g = xTs[s][:, 128 * j : 128 * (j + 1)]
                for q in range(2):
                    n0 = 1024 * h + 512 * q
                    nc.tensor.matmul(
                        out=G[:, 512 * q : 512 * (q + 1)],
                        lhsT=lhsT_g,
                        rhs=xTs[s][:, n0 : n0 + 512],
                        start=True,
                        stop=True,
                    )
                Gs.append(G)
            return Gs

        def emit_exp(Gs, p):
            j, h = p
            Es = []
            for s in range(BPC):
                E = epool.tile([128, 1024], BF16, tag="E", name=f"E_{s}_{j}_{h}")
                nc.scalar.activation(out=E, in_=Gs[s], func=AF.Exp, scale=s2r)
                Es.append(E)
            return Es

        def emit_y(Es, p):
            j, h = p
            for q in range(2):
                n0 = 1024 * h + 512 * q
                for s in range(BPC):
                    nc.tensor.matmul(
                        out=YT[64 * s : 64 * s + 64, n0 : n0 + 512],
                        lhsT=xss[s][:, j, :],
                        rhs=Es[s][:, 512 * q : 512 * (q + 1)],
                        start=(j == 0),
                        stop=(j == NK - 1),
                    )

        Gs = emit_gram(phases[0])
        prev = None
        for i, p in enumerate(phases):
            Es = emit_exp(Gs, p)
            if i + 1 < len(phases):
                Gs = emit_gram(phases[i + 1])
            if prev is not None:
                emit_y(*prev)
            prev = (Es, p)
        emit_y(*prev)

        # ---- epilogue: YT -> natural + combine + store ----
        YTsb = sx.tile([128, T], F32, name="YTsb_both")
        outsbs = [
            sx.tile([128, NK, C], F32, tag=f"outsb{s}", name=f"outsb_{s}")
            for s in range(BPC)
        ]
        ovs = [out_ap[s].rearrange("(p k) c -> p k c", p=128) for s in range(BPC)]
        dma_legs = [nc.sync, nc.gpsimd, nc.scalar, nc.sync]
        for g in range(4):
            nc.vector.tensor_copy(
                out=YTsb[:, 512 * g : 512 * (g + 1)],
                in_=YT[:, 512 * g : 512 * (g + 1)],
            )
            ytr = psG.tile([128, 4, 2 * C], F32, tag="G", name=f"ytr_{g}")
            for kk in range(4):
                k = 4 * g + kk
                nc.tensor.transpose(
                    out=ytr[:, kk, :],
                    in_=YTsb[:, 128 * k : 128 * (k + 1)],
                    identity=ident,
                )
            for s in range(BPC):
                for kk in range(4):
                    k = 4 * g + kk
                    nc.vector.scalar_tensor_tensor(
                        out=outsbs[s][:, k, :],
                        in0=ytr[:, kk, 64 * s : 64 * s + 64],
                        scalar=fs[s][:, k : k + 1],
                        in1=ax2[:, k, 64 * s : 64 * s + 64],
                        op0=mybir.AluOpType.mult,
                        op1=mybir.AluOpType.add,
                    )
            for s in range(BPC):
                dma_legs[(2 * g + s) % 4].dma_start(
                    out=ovs[s][:, 4 * g : 4 * (g + 1), :],
                    in_=outsbs[s][:, 4 * g : 4 * (g + 1), :],
                )


_NC_CACHE = {}


def _get_nc(reps=1, stages='all'):
    key = (reps, stages)
    if key not in _NC_CACHE:
        _NC_CACHE[key] = build_nc(reps, stages)
    return _NC_CACHE[key]


def _run(x, r_sigma, margin, trace=False, reps=1, stages='all'):
    nc = _get_nc(reps, stages)
    x = np.ascontiguousarray(np.asarray(x, dtype=np.float32))
    r_sigma = np.ascontiguousarray(np.asarray(r_sigma, dtype=np.float32))
    margin = np.ascontiguousarray(np.asarray(margin, dtype=np.float32))
    in_maps = [
        {
            "x": np.ascontiguousarray(x[c * BPC : (c + 1) * BPC]),
            "r_sigma": r_sigma,
            "margin": margin,
        }
        for c in range(N_CORES)
    ]
    res = run_bass_kernel_spmd(nc, in_maps, core_ids=list(range(N_CORES)), trace=trace)
    out = np.concatenate([res.results[c]["out"] for c in range(N_CORES)], axis=0)
    return out, res


def kernel(x, r_sigma, margin):
    out, _ = _run(x, r_sigma, margin, trace=False)
    return out
